# revision 7
# baseline (speedup 1.0000x reference)
"""AGREE group-recommendation forward pass on 8 TRN2 NeuronCores.

Data-parallel over the B=1M batch. Host does index-space work only:
global item-major sort shards contiguous item ranges to cores (item
table slices fit int16); within a core, elements sort by (group, item)
and split into two exact-65536 streams with OVERLAPPING group-table
slices (baseA/baseB chosen per core), so there is zero slot padding.

Device: per 4096-element block, two dma_gathers fetch 256B rows
(group row: me3|G1|const1, item row: ie|I1|0|I2), then the MLP runs
at whole-block granularity: 10 DVE ops + 4 ACT ops per block, DMA-xbar
transposes (no PE transposes), and 14 batched PE matmuls per block.
sigmoid via 0.5+0.5*tanh(x/2); host applies the affine + unscramble.
"""

import sys

sys.path.insert(0, "/opt/trn_rl_repo")

import numpy as np

import bass_rust
import concourse.bass as bass
import concourse.library_config as library_config
import concourse.mybir as mybir
import concourse.tile as tile_mod
from concourse.bass_utils import run_bass_kernel_spmd
from concourse.vector_clock import ScopedClock

NCORES = 8
B = 1048576
N = B // NCORES          # 131072 elements per core
BLK = 4096               # elements per block
NBLK = 32                # blocks per core (no padding)
STRB = 16                # blocks per group-half stream
SSLOT = STRB * BLK       # 65536 elements per stream
J = 32                   # 128-element j-slots per block
D = 32
NG = 50000
NGT = 32768              # rows per (overlapping) group table slice
NU = 200000
NI = 100000
MAXI = 16384             # per-core item table rows
IDXC = BLK // 16         # 256 idx columns per block

F32 = mybir.dt.float32
F16 = mybir.dt.float16
I16 = mybir.dt.int16
AF = mybir.ActivationFunctionType
MUL = mybir.AluOpType.mult
ADD = mybir.AluOpType.add

BENCH = {"trace": False}

# ---------------------------------------------------------------------------
# The neuronxcc in this container rejects instructions carrying >2 sync
# waits (CoreV3 setupSyncWait). Tile's end-of-context drain waits on the
# whole global clock in one instruction; split those waits across SP nops.
_MAXW = 1


def _patched_drain_and_barrier(self, tick_clock, wait_clock):
    probe = self.nc.sync.nop(nofuse=True, hint="drain_wait_split")
    wait_clock.add_sem_waits(probe.ins, ScopedClock({None: tick_clock.global_clock}))
    si = probe.ins.sync_info
    waits = list(si.on_wait) if si is not None else []
    ups = list(si.on_update) if si is not None else []
    probe.ins.sync_info = bass_rust.SyncInfo(on_wait=waits[:_MAXW], on_update=ups)
    for i in range(_MAXW, len(waits), _MAXW):
        n = self.nc.sync.nop(nofuse=True, hint="drain_wait_split")
        n.ins.sync_info = bass_rust.SyncInfo(
            on_wait=waits[i : i + _MAXW], on_update=[]
        )
    self.nc.sync.drain()
    self.nc.all_engine_barrier()
    assert self.sems is not None
    popped = self.nc._tile_sem_poison_stack.pop()
    assert popped is self._sem_poison
    self.nc.clear_and_free_semaphores(list(self.sems.allocated().values()))
    self.nc.all_engine_barrier()


tile_mod.TileContext._drain_and_barrier = _patched_drain_and_barrier


def _split_sync_waits(nc, max_waits=1):
    """Post-pass: no instruction may carry more than max_waits sem waits
    (neuronxcc setupSyncWait limit). Move excess waits onto preceding
    same-engine nops."""
    cnt = 0
    for f in nc.m.functions:
        for bb in f.blocks:
            out = []
            changed = False
            for inst in bb.instructions:
                si = inst.sync_info
                if si is not None and len(si.on_wait) > max_waits:
                    waits = list(si.on_wait)
                    ncarry = len(waits) - max_waits
                    for k in range(0, ncarry, max_waits):
                        cnt += 1
                        out.append(mybir.InstNoOp(
                            name=f"waitsplit-{cnt}",
                            engine=inst.engine,
                            bass_nofuse=True,
                            sync_info=mybir.SyncInfo(
                                on_wait=waits[k : k + max_waits], on_update=[]
                            ),
                        ))
                    inst.sync_info = mybir.SyncInfo(
                        on_wait=waits[ncarry:], on_update=list(si.on_update)
                    )
                    changed = True
                out.append(inst)
            if changed:
                bb.instructions = out
    return cnt
# ---------------------------------------------------------------------------


def build_program(split_waits=True, blocks=None, finalize=True):
    nc = bass.Bass(num_swdge_queues=4)
    gx_ext = nc.declare_dram_parameter("gx", [128, NBLK * IDXC], I16, isOutput=False)
    ix_ext = nc.declare_dram_parameter("ix", [128, NBLK * IDXC], I16, isOutput=False)
    mea_ext = nc.declare_dram_parameter("mea", [NGT, 128], F16, isOutput=False)
    meb_ext = nc.declare_dram_parameter("meb", [NGT, 128], F16, isOutput=False)
    ib_ext = nc.declare_dram_parameter("ib", [MAXI, 128], F16, isOutput=False)
    w2r_ext = nc.declare_dram_parameter("w2r", [128, 51], F16, isOutput=False)
    bda_ext = nc.declare_dram_parameter("bda", [128, 32], F16, isOutput=False)
    bdb_ext = nc.declare_dram_parameter("bdb", [128, 32], F16, isOutput=False)
    bdf_ext = nc.declare_dram_parameter("bdf", [32, 4], F16, isOutput=False)
    pb1_ext = nc.declare_dram_parameter("pb1", [32], F32, isOutput=False)
    pb2_ext = nc.declare_dram_parameter("pb2", [4], F32, isOutput=False)
    idn_ext = nc.declare_dram_parameter("idn", [128, 128], F16, isOutput=False)
    out_ext = nc.declare_dram_parameter("out", [N], F32, isOutput=True)

    with tile_mod.TileContext(nc) as tc:
        with (
            tc.tile_pool(name="const", bufs=1) as cp,
            tc.tile_pool(name="io", bufs=5) as io,
            tc.tile_pool(name="comp", bufs=3) as co,
            tc.tile_pool(name="psh", bufs=4, space="PSUM") as ps_h,
            tc.tile_pool(name="psy", bufs=2, space="PSUM") as ps_y,
        ):
            nreg = nc.gpsimd.to_reg(BLK // 2)

            w2rsb = cp.tile([128, 51], F16)
            nc.sync.dma_start(out=w2rsb[:], in_=w2r_ext[:])
            bdasb = cp.tile([128, 32], F16)
            nc.sync.dma_start(out=bdasb[:], in_=bda_ext[:])
            bdbsb = cp.tile([128, 32], F16)
            nc.sync.dma_start(out=bdbsb[:], in_=bdb_ext[:])
            bdfsb = cp.tile([32, 4], F16)
            nc.sync.dma_start(out=bdfsb[:], in_=bdf_ext[:])
            pb1sb = cp.tile([32, 1], F32)
            nc.sync.dma_start(out=pb1sb[:], in_=pb1_ext[:, None])
            pb2sb = cp.tile([4, 1], F32)
            nc.sync.dma_start(out=pb2sb[:], in_=pb2_ext[:, None])
            idn16 = cp.tile([128, 128], F16)
            nc.sync.dma_start(out=idn16[:], in_=idn_ext[:])

            gxsb = cp.tile([128, NBLK * IDXC], I16)
            nc.sync.dma_start(out=gxsb[:], in_=gx_ext[:])
            ixsb = cp.tile([128, NBLK * IDXC], I16)
            nc.sync.dma_start(out=ixsb[:], in_=ix_ext[:])

            def body(b):
                mtbl = mea_ext if b < STRB else meb_ext
                # Four half-gathers per block across all 4 SWDGE queues (one
                # per Q7 core pair / descriptor ring) so all 16 SDMA engines
                # drain 4 rings concurrently; single_packet concatenates each
                # engine's read stream to amortize per-packet overhead.
                HB = BLK // 2            # 2048 idxs per half-gather
                HC = IDXC // 2
                recA = io.tile([128, BLK], F16, tag="recA")
                recB = io.tile([128, BLK], F16, tag="recB")
                for hh in range(2):
                    nc.gpsimd.dma_gather(
                        recA[:, hh * HB : (hh + 1) * HB].rearrange(
                            "p (j r) -> p j r", r=128
                        ),
                        mtbl[:],
                        gxsb[:, b * IDXC + hh * HC : b * IDXC + (hh + 1) * HC],
                        HB,
                        nreg,
                        128,
                        single_packet=False,
                        queue_num=2 * hh,
                    )
                    nc.gpsimd.dma_gather(
                        recB[:, hh * HB : (hh + 1) * HB].rearrange(
                            "p (j r) -> p j r", r=128
                        ),
                        ib_ext[:],
                        ixsb[:, b * IDXC + hh * HC : b * IDXC + (hh + 1) * HC],
                        HB,
                        nreg,
                        128,
                        single_packet=False,
                        queue_num=2 * hh + 1,
                    )

                # element (p, j): recA[p,j,:] = me0|me1|me2|G1'(17 incl 1.0)|pad
                #                 recB[p,j,:] = ie|I1'(17 incl 0)|I2|pad
                rA = recA[:].rearrange("p (j r) -> p j r", r=128)
                rAmo = recA[:].rearrange("p (j m d) -> p m j d", m=4, d=D)
                rB = recB[:].rearrange("p (j r) -> p j r", r=128)

                # h = relu(G1' + I1')  [128, J*17] f16 (17th col = softmax
                # bias channel: 1.0 + 0.0 -> relu -> 1.0, weighted by b2 row)
                hel = co.tile([128, J * 17], F16, tag="hel")
                hel_v = hel[:].rearrange("p (j k) -> p j k", k=17)
                nc.vector.tensor_tensor(
                    out=hel_v, in0=rA[:, :, 96:113], in1=rB[:, :, 32:49], op=ADD
                )
                nc.scalar.activation(out=hel[:], in_=hel[:], func=AF.Relu)

                # logits+b2 = h @ w2e, element-major broadcast mult + reduce
                lprod = co.tile([128, J * 51], F16, tag="lprod")
                lprod_v = lprod[:].rearrange("p (j m k) -> p j m k", m=3, k=17)
                nc.vector.tensor_tensor(
                    out=lprod_v,
                    in0=hel_v.unsqueeze(2).to_broadcast([128, J, 3, 17]),
                    in1=w2rsb[:].rearrange("p (m k) -> p m k", m=3)
                    .unsqueeze(1).to_broadcast([128, J, 3, 17]),
                    op=MUL,
                )
                sts = co.tile([128, J * 3], F32, tag="sts")
                st_v = sts[:].rearrange("p (j m) -> p j m", m=3)
                nc.vector.tensor_reduce(
                    out=st_v, in_=lprod_v, axis=mybir.AxisListType.X, op=ADD
                )
                nc.scalar.activation(out=sts[:], in_=sts[:], func=AF.Exp)
                dsum = co.tile([128, J], F32, tag="dsum")
                nc.vector.tensor_reduce(
                    out=dsum[:], in_=st_v, axis=mybir.AxisListType.X, op=ADD
                )
                rsb = co.tile([128, J], F32, tag="rsb")
                nc.vector.reciprocal(out=rsb[:], in_=dsum[:])
                wt = co.tile([128, J * 3], F16, tag="wt")
                wt_v = wt[:].rearrange("p (j m) -> p j m", m=3)
                nc.vector.tensor_tensor(
                    out=wt_v,
                    in0=st_v,
                    in1=rsb[:].unsqueeze(2).to_broadcast([128, J, 3]),
                    op=MUL,
                )
                # g = sum_m wt_m * me_m (m-major so the reduction is two adds)
                prod = co.tile([128, 3 * J * D], F16, tag="prod")
                prod_v = prod[:].rearrange("p (m j d) -> p m j d", m=3, d=D)
                nc.vector.tensor_tensor(
                    out=prod_v,
                    in0=rAmo[:, 0:3],
                    in1=wt[:].rearrange("p (j m) -> p m j", m=3)
                    .unsqueeze(3).to_broadcast([128, 3, J, D]),
                    op=MUL,
                )
                gpart = co.tile([128, J * D], F16, tag="gpart")
                nc.vector.tensor_tensor(
                    out=gpart[:], in0=prod[:, 0 : J * D],
                    in1=prod[:, J * D : 2 * J * D], op=ADD,
                )
                g = co.tile([128, J * D], F16, tag="g")
                g_v = g[:].rearrange("p (j d) -> p j d", d=D)
                nc.vector.tensor_tensor(
                    out=g[:], in0=gpart[:], in1=prod[:, 2 * J * D : 3 * J * D],
                    op=ADD,
                )
                gie = co.tile([128, J * D], F16, tag="gie")
                gie_v = gie[:].rearrange("p (j d) -> p j d", d=D)
                nc.vector.tensor_tensor(
                    out=gie_v, in0=g_v, in1=rB[:, :, 0:32], op=MUL
                )
                # contiguous copy of host-precomputed I2 = ie @ C
                i2c = co.tile([128, J * 8], F16, tag="i2c")
                i2c_v = i2c[:].rearrange("p (j k) -> p j k", k=8)
                nc.scalar.activation(out=i2c_v, in_=rB[:, :, 49:57], func=AF.Copy)

                # feature-major via DMA xbar transpose (batched 8x128 tiles):
                # giepT[pfeat, h, e] = gie[e, 128*h + pfeat]
                giepT = co.tile([128, 8 * 128], F16, tag="giepT")
                nc.sync.dma_start_transpose(
                    out=giepT[:].rearrange("p (h e) -> p h e", e=128),
                    in_=gie[:],
                )
                gpT = co.tile([128, 8 * 128], F16, tag="gpT")
                nc.sync.dma_start_transpose(
                    out=gpT[:].rearrange("p (h e) -> p h e", e=128),
                    in_=g[:],
                )

                # h2 = relu(A@gieT + B@gT + I2T + b1), two 512-col psum groups
                h2sb = co.tile([32, 1024], F16, tag="h2sb")
                for G in range(2):
                    h2_ps = ps_h.tile([32, 512], F32, tag="h2ps")
                    # one start over the full bank, then pure accumulation
                    # (multiple start=True writes re-zero the whole bank)
                    nc.tensor.matmul(
                        out=h2_ps[:], lhsT=bdasb[:],
                        rhs=giepT[:, 512 * G : 512 * (G + 1)],
                        start=True, stop=False,
                    )
                    nc.tensor.matmul(
                        out=h2_ps[:], lhsT=bdbsb[:],
                        rhs=gpT[:, 512 * G : 512 * (G + 1)],
                        start=False, stop=False,
                    )
                    for t in range(4):
                        nc.tensor.matmul(
                            out=h2_ps[:, 128 * t : 128 * (t + 1)],
                            lhsT=i2c[:, 32 * (4 * G + t) : 32 * (4 * G + t) + 32],
                            rhs=idn16[:],
                            start=False, stop=(t == 3),
                        )
                    nc.scalar.activation(
                        out=h2sb[:, 512 * G : 512 * (G + 1)], in_=h2_ps[:],
                        func=AF.Relu, bias=pb1sb[:],
                    )

                # y = P2 @ h2 (block-diag over 4 j-phases), then tanh
                y_ps = ps_y.tile([4, 1024], F32, tag="yps")
                for G in range(2):
                    nc.tensor.matmul(
                        out=y_ps[:, 512 * G : 512 * (G + 1)], lhsT=bdfsb[:],
                        rhs=h2sb[:, 512 * G : 512 * (G + 1)],
                        start=True, stop=True,
                    )
                ysb = co.tile([4, 1024], F32, tag="ysb")
                # sigmoid(x) = 0.5 + 0.5*tanh(x/2); pb2 pre-halved on host
                nc.scalar.activation(
                    out=ysb[:], in_=y_ps[:], func=AF.Tanh,
                    bias=pb2sb[:], scale=0.5,
                )
                nc.sync.dma_start(
                    out=out_ext[bass.ts(b, BLK)].rearrange("(jj c) -> jj c", jj=4),
                    in_=ysb[:],
                )

            for b in (range(NBLK) if blocks is None else blocks):
                body(b)

    if split_waits:
        _split_sync_waits(nc)
    if finalize:
        # dma_gather needs the gpsimd "mlp" ucode library resident: insert
        # the library-reload instructions and lower them to encoded ISA form
        # (the two passes Bacc.compile runs; plain Bass skips them).
        inst_type_to_lib_mask = {}
        for lib in library_config.all_libraries:
            for t in lib.instructions:
                inst_type_to_lib_mask[t] = (
                    inst_type_to_lib_mask.get(t, 0) | (1 << lib.index)
                )
        bass_rust.insert_library_loads(
            nc, inst_type_to_lib_mask, len(library_config.all_libraries),
            library_config.standard.index,
        )
        mybir.codegen_inst_isa_subclasses(nc)
    return nc


_prog_cache = {}


def _get_program():
    if "p" not in _prog_cache:
        _prog_cache["p"] = build_program()
    return _prog_cache["p"]


def _bd(p1part):
    out = np.zeros([128, 32], dtype=np.float32)
    for jj in range(4):
        out[32 * jj : 32 * (jj + 1), 8 * jj : 8 * (jj + 1)] = p1part
    return out.astype(np.float16)


def _bdf(p2):
    out = np.zeros([32, 4], dtype=np.float32)
    for jj in range(4):
        out[8 * jj : 8 * (jj + 1), jj] = p2.reshape(-1)
    return out.astype(np.float16)


def _idx_dev_layout(vals):
    """[NBLK*BLK] int16 slot values -> [128, NBLK*IDXC] device idx layout:
    index i of block b at [i % 16, b*IDXC + i // 16], replicated across the
    8 groups of 16 partitions."""
    v = vals.reshape(NBLK, IDXC, 16)            # i = col*16 + row
    v = np.ascontiguousarray(v.transpose(0, 2, 1))  # [NBLK, 16, IDXC]
    flat = v.transpose(1, 0, 2).reshape(16, NBLK * IDXC)
    return np.ascontiguousarray(np.tile(flat, (8, 1)))


def _out_perm():
    """Device output position o(s) for gather slot s: element at slot
    (b, j, e) lands at out[b*4096 + (j%4)*1024 + (j//16)*512 +
    ((j//4)%4)*128 + e]."""
    s = np.arange(N)
    b = s >> 12
    r = s & 4095
    j = r >> 7
    e = r & 127
    return (b << 12) + ((j & 3) << 10) + ((j >> 4) << 9) + (((j >> 2) & 3) << 7) + e


def prep_inputs(group_inputs, item_inputs, group_members, user_emb, item_emb,
                att_w1, att_b1, att_w2, att_b2,
                pred_w1, pred_b1, pred_w2, pred_b2):
    gm = np.asarray(group_members, dtype=np.int64)
    ue = np.asarray(user_emb, dtype=np.float32)
    w1 = np.asarray(att_w1, dtype=np.float32)
    b1v = np.asarray(att_b1, dtype=np.float32)
    w2 = np.asarray(att_w2, dtype=np.float32)
    b2v = np.asarray(att_b2, dtype=np.float32)
    iemb = np.asarray(item_emb, dtype=np.float32)
    me3f = ue[gm].reshape(NG, 3 * D)
    g1 = me3f @ w1[0:96] + b1v
    i1 = iemb @ w1[96:128]
    tbl_g = np.zeros([NG, 128], dtype=np.float16)
    tbl_g[:, 0:96] = me3f
    tbl_g[:, 96:112] = g1
    tbl_g[:, 112] = 1.0                       # softmax-bias channel
    p1f = np.asarray(pred_w1, dtype=np.float32)
    i2 = iemb @ p1f[64:96]
    tbl_i = np.zeros([NI, 128], dtype=np.float16)
    tbl_i[:, 0:32] = iemb
    tbl_i[:, 32:48] = i1                      # col 48 stays 0 (bias channel)
    tbl_i[:, 49:57] = i2
    # w2 extended with b2 as a 17th row, m-major flat [3*17]
    w2e = np.concatenate([w2, b2v.reshape(1, 3)], axis=0)  # [17, 3]
    common = {
        "w2r": np.ascontiguousarray(
            np.broadcast_to(w2e.T.reshape(1, 51), (128, 51)).astype(np.float16)),
        "bda": _bd(p1f[0:32]),
        "bdb": _bd(p1f[32:64]),
        "bdf": _bdf(np.asarray(pred_w2, dtype=np.float32)),
        "pb1": np.tile(np.asarray(pred_b1, dtype=np.float32), 4),
        "pb2": np.full([4], 0.5 * np.asarray(pred_b2,
                       dtype=np.float32).reshape(-1)[0], dtype=np.float32),
        "idn": np.eye(128, dtype=np.float16),
    }

    gi = np.asarray(group_inputs, dtype=np.int64)
    it = np.asarray(item_inputs, dtype=np.int64)
    order = np.argsort(it * NG + gi, kind="stable")  # item-major global sort
    operm = _out_perm()

    in_maps = []
    placements = []  # (orig_positions_in_slot_order, out_perm)
    for c in range(NCORES):
        sl = order[c * N : (c + 1) * N]
        gic = gi[sl]
        itc = it[sl]
        it_lo = int(itc.min())
        span = int(itc.max()) - it_lo + 1
        assert span <= MAXI, f"core {c}: item span {span} > {MAXI}"
        ibt = np.zeros([MAXI, 128], dtype=np.float16)
        ibt[:span] = tbl_i[it_lo : it_lo + span]

        # per-core (group, item) sort; exact halves with overlapping tables
        so = np.argsort(gic * NI + itc, kind="stable")
        sel = sl[so]
        gs = gic[so]
        its = itc[so]
        baseA = int(gs[0])
        spanA = int(gs[SSLOT - 1]) - baseA + 1
        baseB = int(gs[SSLOT])
        spanB = int(gs[-1]) - baseB + 1
        assert spanA <= NGT and spanB <= NGT, (c, spanA, spanB)
        mea = np.zeros([NGT, 128], dtype=np.float16)
        mea[:spanA] = tbl_g[baseA : baseA + spanA]
        meb = np.zeros([NGT, 128], dtype=np.float16)
        meb[:spanB] = tbl_g[baseB : baseB + spanB]

        gvals = np.empty(N, dtype=np.int16)
        gvals[:SSLOT] = (gs[:SSLOT] - baseA).astype(np.int16)
        gvals[SSLOT:] = (gs[SSLOT:] - baseB).astype(np.int16)
        ivals = (its - it_lo).astype(np.int16)

        m = dict(common)
        m["mea"] = mea
        m["meb"] = meb
        m["ib"] = ibt
        m["gx"] = _idx_dev_layout(gvals)
        m["ix"] = _idx_dev_layout(ivals)
        in_maps.append(m)
        placements.append((sel, operm))
    return in_maps, placements


def kernel(**inputs):
    nc = _get_program()
    in_maps, placements = prep_inputs(**inputs)
    res = run_bass_kernel_spmd(
        nc, in_maps, core_ids=list(range(NCORES)), trace=BENCH.get("trace", False)
    )
    BENCH["last_result"] = res
    out = np.empty(B, dtype=np.float32)
    for c in range(NCORES):
        # device returns tanh((h2@P2+b2)/2); sigmoid = 0.5 + 0.5*tanh
        y = 0.5 + 0.5 * np.asarray(res.results[c]["out"]).reshape(-1)
        sel, operm = placements[c]
        out[sel] = y[operm]
    return out.reshape(B, 1).astype(np.float32)
